# revision 10
# baseline (speedup 1.0000x reference)
"""ANI-style AEV computer (radial + angular) on 8 Trainium2 NeuronCores.

Strategy
--------
Data-parallel over molecules (32/core), host-side *indexing only*; all AEV
float math runs on-device.

Angular: host enumerates surviving triples (center i, neighbors j<k within
Rca) into a flat per-core list sorted by (slot, half, center, pair-bin).
Device computes geometry -> cutoffs -> f2/f1 -> G (bf16, 32 feats) per
128-triple chunk and bins G into (center, species-pair) segments with PE
matmuls against per-chunk one-hot matrices (fp8 stationary, PSUM-accum
per group).  The feature chain is split into two chunk-halves so the PE
binning of half 0 overlaps the DVE/ACT work of half 1.

Radial: dense over all (i,j) pairs, 768 rows packed as 6x128 partitions,
species-binned with small block one-hot matmuls packed 4-up into a PSUM
bank via tile_position col-tiling.

Engine split: DVE runs every two-input elementwise op; ACT runs every
unary op (ln/exp/square, with scale+bias folds such as sin via
ln(1-cos^2)).  GpSimd is NOT used: its SBUF port is the same physical
port DVE needs for two-input ops (exclusive full-instruction lock).
Self-pair masking is done by offsetting diagonal coordinates host-side.
Inputs arrive as TWO contiguous DMAs (f32 pack on the ACT queue, fp8
pack on the sync queue) to minimize serial descriptor generation.  One
ACT table set serves every activation.  Outputs in bf16.
"""

import os
import sys

import numpy as np

for _p in ("/opt/trn_rl_repo", "/root/.axon_site/_ro/trn_rl_repo"):
    if os.path.isdir(_p) and _p not in sys.path:
        sys.path.insert(0, _p)

import concourse.bass as bass
import concourse.mybir as mybir
from concourse import bacc, tile
from concourse.bass_utils import run_bass_kernel_spmd

import ml_dtypes

AF = mybir.ActivationFunctionType
ALU = mybir.AluOpType
dt = mybir.dt
AP = bass.AP
FP8 = ml_dtypes.float8_e4m3

# ---- hyperparameters (match reference) ----
NCORES = 8
M, A = 256, 24
MLOC = M // NCORES          # 32 molecules per core
RCR, RCA = 5.2, 3.5
ETA_R, ETA_A, ZETA = 16.0, 8.0, 32.0
SHF_R = np.linspace(0.9, 5.2, 17)[:-1].astype(np.float64)   # 16
SHF_A = np.linspace(0.9, 3.5, 5)[:-1].astype(np.float64)    # 4
SHF_Z = (np.arange(8) + 0.5) * np.pi / 8.0                   # 8
NPAIR, RSUB, ASUB = 10, 16, 32
NSEG = 120
GSEG = 128                  # one-hot width
NG = 2 * MLOC               # 64 groups/core
PGRP = 16                   # psum groups per PSUM bank tile
RG = 6                      # radial groups: 768 rows = 6 x 128
RSEGW = 32                  # radial one-hot width per group

_TRIU = np.zeros((4, 4), np.int64)
_s1, _s2 = np.triu_indices(4)
_TRIU[_s1, _s2] = np.arange(len(_s1))
_TRIU[_s2, _s1] = _TRIU[_s1, _s2]

# ---- degree-4 (in v=u^2) Chebyshev fit of cos(pi*u/2) on u in [0,1] ----
def _cos_poly():
    v = np.linspace(0.0, 1.0, 4001)
    tgt = np.cos(0.5 * np.pi * np.sqrt(v))
    from numpy.polynomial import chebyshev as C
    ch = C.Chebyshev.fit(v, tgt, 4, domain=[0, 1])
    pw = ch.convert(kind=np.polynomial.Polynomial)
    c = pw.coef
    K = c[4]
    a = c[:4] / K
    return K, a

_POLY_K, _POLY_A = _cos_poly()

# const column map (first 33 cols of the f32 input pack)
_C_SHF2A = 0     # 4  : 2*shf_a
_C_SHFR = 4      # 16 : shf_r
_C_CZSZ = 20     # 8  : 0.5*cos(shf_z[0:4]) | 0.5*sin(shf_z[0:4])
_C_F2B = 28      # 1  : angular exp bias ln(2*K^4)
_C_RADB = 29     # 1  : radial exp bias ln(0.25*K^2)
_C_HALF = 30     # 1  : 0.5
_C_NLRCR = 31    # 1  : -ln(RCR)
_C_ONE = 32      # 1  : 1.0
_C_W = 33


def _build_consts():
    ct = np.zeros((128, _C_W), np.float32)
    ct[:, _C_SHF2A:_C_SHF2A + 4] = 2.0 * SHF_A
    ct[:, _C_SHFR:_C_SHFR + 16] = SHF_R
    ct[:, _C_CZSZ:_C_CZSZ + 4] = 0.5 * np.cos(SHF_Z[:4])
    ct[:, _C_CZSZ + 4:_C_CZSZ + 8] = 0.5 * np.sin(SHF_Z[:4])
    K = _POLY_K
    ct[:, _C_F2B] = np.log(2.0) + 4.0 * np.log(abs(K))
    ct[:, _C_RADB] = np.log(0.25) + 2.0 * np.log(abs(K))
    ct[:, _C_HALF] = 0.5
    ct[:, _C_NLRCR] = -np.log(RCR)
    ct[:, _C_ONE] = 1.0
    return ct


# ============================================================
# host-side indexing prep
# ============================================================

def _prep(species, coordinates):
    sp = np.asarray(species)
    co = np.asarray(coordinates, np.float32)
    cod = co.astype(np.float64)
    vec = cod[:, None, :, :] - cod[:, :, None, :]
    dmat = np.sqrt(np.maximum((vec ** 2).sum(-1), 0.0))
    adj = (dmat <= RCA) & ~np.eye(A, dtype=bool)[None]

    nbrs = [[np.where(adj[m, i])[0] for i in range(A)] for m in range(M)]
    tri_mi = np.array([[len(nbrs[m][i]) * (len(nbrs[m][i]) - 1) // 2
                        for i in range(A)] for m in range(M)], np.int64)
    Th = np.stack([tri_mi[:, :12].sum(1), tri_mi[:, 12:].sum(1)], 1)

    order = np.argsort(-(Th.sum(1)), kind="stable")
    slot2mol = np.empty((NCORES, MLOC), np.int64)
    for s in range(MLOC):
        for c in range(NCORES):
            slot2mol[c, s] = order[s * NCORES + c]

    SYNCW = 4
    tlo = np.zeros((NCORES, NG), np.int64)
    thi = np.zeros((NCORES, NG), np.int64)
    posv = np.zeros(NCORES, np.int64)
    for g in range(NG):
        s, h = g // 2, g % 2
        if g % SYNCW == 0:
            posv[:] = int(np.ceil(posv.max() / 128.0)) * 128
        tlo[:, g] = posv
        posv += Th[slot2mol[:, s], h]
        thi[:, g] = posv
    nch = int(np.ceil(posv.max() / 128.0))
    clo = np.empty(NG, np.int64)
    chi = np.empty(NG, np.int64)
    for g in range(NG):
        clo[g] = (tlo[:, g] // 128).min()
        hi = np.maximum(thi[:, g] - 1, tlo[:, g]) // 128
        chi[g] = max(hi.max(), clo[g])
    span = (chi - clo + 1).astype(np.int64)
    mm_base = np.concatenate([[0], np.cumsum(span)])
    n_mm = int(mm_base[-1])

    # f32 input pack: [consts(33) | pj|pk|ci (9*nch) | radf (450)]
    NF = _C_W + 9 * nch + RG * 75
    f32in = np.zeros((NCORES, 128, NF), np.float32)
    f32in[:, :, :_C_W] = _build_consts()
    o = _C_W
    pj = f32in[:, :, o:o + 3 * nch].reshape(NCORES, 128, nch, 3)
    pk = f32in[:, :, o + 3 * nch:o + 6 * nch].reshape(NCORES, 128, nch, 3)
    ci = f32in[:, :, o + 6 * nch:o + 9 * nch].reshape(NCORES, 128, nch, 3)
    o += 9 * nch
    rcj = f32in[:, :, o:o + RG * 3].reshape(NCORES, 128, RG, 3)
    rcb = f32in[:, :, o + RG * 3:].reshape(NCORES, 128, RG, 3, 24)

    # fp8 input pack: [rsp (RG*32) | oh (n_mm*128)]
    bf8in = np.zeros((NCORES, 128, RG * RSEGW + n_mm * GSEG), FP8)
    rsp = bf8in[:, :, :RG * RSEGW]
    oh = bf8in[:, :, RG * RSEGW:].reshape(NCORES, 128, n_mm, GSEG)

    for c in range(NCORES):
        def put_pad(a, b, mref):
            if a >= b:
                return
            t_idx = np.arange(a, b)
            chs, ts = t_idx // 128, t_idx % 128
            pj[c, ts, chs] = mref + np.array([50, 0, 0], np.float32)
            pk[c, ts, chs] = mref + np.array([0, 50, 0], np.float32)
            ci[c, ts, chs] = mref
        prev_end = 0
        for s in range(MLOC):
            m = slot2mol[c, s]
            for h in range(2):
                g = 2 * s + h
                put_pad(prev_end, tlo[c, g], co[m, 0])
                pos = tlo[c, g]
                for u in range(12):
                    i = h * 12 + u
                    nb = nbrs[m][i]
                    if len(nb) < 2:
                        continue
                    jj, kk = np.triu_indices(len(nb), 1)
                    j, k = nb[jj], nb[kk]
                    p = _TRIU[sp[m, j], sp[m, k]]
                    o2 = np.argsort(p, kind="stable")
                    j, k, p = j[o2], k[o2], p[o2]
                    n = len(j)
                    t_idx = np.arange(pos, pos + n)
                    chs, ts = t_idx // 128, t_idx % 128
                    pj[c, ts, chs] = co[m, j]
                    pk[c, ts, chs] = co[m, k]
                    ci[c, ts, chs] = np.broadcast_to(co[m, i], (n, 3))
                    oh[c, ts, mm_base[g] + chs - clo[g], p * 12 + u] = 1
                    pos += n
                prev_end = pos
        put_pad(prev_end, nch * 128, co[slot2mol[c, 0], 0])

    s0g = [(128 * g) // 24 for g in range(RG)]
    for c in range(NCORES):
        for g in range(RG):
            for p in range(128):
                r = g * 128 + p
                s, j = r // 24, r % 24
                m = slot2mol[c, s]
                rcj[c, p, g] = co[m, j]
                cb = co[m].T.copy()
                cb[:, j] += 10.0                 # self-pair offset
                rcb[c, p, g] = cb
                rsp[c, p, g * RSEGW + (s - s0g[g]) * 4 + sp[m, j]] = 1

    meta = dict(nch=nch, n_mm=n_mm, clo=tuple(int(x) for x in clo),
                chi=tuple(int(x) for x in chi), slot2mol=slot2mol,
                s0g=s0g)
    arrays = dict(f32in=f32in, bf8in=bf8in)
    return meta, arrays


# ============================================================
# device program
# ============================================================

def _bb(ap, dims, off=0):
    return AP(ap.tensor, ap.offset + off,
              [list(ap.ap[0])] + [list(d) for d in dims])


def _build(nch, clo, chi):
    span = [chi[g] - clo[g] + 1 for g in range(NG)]
    mm_base = [0]
    for g in range(NG):
        mm_base.append(mm_base[-1] + span[g])
    n_mm = mm_base[-1]
    nb = nch
    NF = _C_W + 9 * nch + RG * 75
    NB8 = RG * RSEGW + n_mm * GSEG

    nc = bacc.Bacc(None, target_bir_lowering=False)
    f32_d = nc.declare_dram_parameter("f32in", [128, NF], dt.float32, False)
    bf8_d = nc.declare_dram_parameter("bf8in", [128, NB8], dt.float8e4,
                                      False)
    outa_d = nc.declare_dram_parameter("outa", [GSEG, NG * 32], dt.bfloat16,
                                       True)
    outr_d = nc.declare_dram_parameter("outr", [128, 2 * 24 * 16],
                                       dt.bfloat16, True)

    a = _POLY_A
    f32, bf16 = dt.float32, dt.bfloat16

    with tile.TileContext(nc) as tc:
        with (
            tc.tile_pool(name="io", bufs=1) as io,
            tc.tile_pool(name="geo", bufs=1) as geo,
            tc.tile_pool(name="feat", bufs=1) as feat,
            tc.tile_pool(name="stg", bufs=1) as stg,
            tc.tile_pool(name="gp", bufs=3) as gp,
            tc.tile_pool(name="ps", bufs=4, space="PSUM") as ps,
            tc.tile_pool(name="psr", bufs=2, space="PSUM") as psr,
        ):
            V = nc.vector
            S = nc.scalar

            F32 = io.tile([128, NF], f32, tag="f32in")
            S.dma_start(F32[:], f32_d[:])
            B8 = io.tile([128, NB8], dt.float8e4, tag="bf8in")
            nc.sync.dma_start(B8[:], bf8_d[:])

            CT = F32
            AOFF = _C_W
            ROFF = _C_W + 9 * nch
            PJPK = F32[:, AOFF:AOFF + 6 * nch]
            CI = F32[:, AOFF + 6 * nch:AOFF + 9 * nch]
            CJ = F32[:, ROFF:ROFF + RG * 3]
            CB = F32[:, ROFF + RG * 3:ROFF + RG * 75]
            RSP = B8[:, 0:RG * RSEGW]
            OHO = RG * RSEGW

            AZSTG = stg.tile([GSEG, NG * 32], bf16)
            RDSTG = stg.tile([128, 768], bf16)

            def bias(col):
                return CT[:, col:col + 1]

            # ================= geometry (V + S) =================
            VJK = geo.tile([128, 6 * nb], f32, tag="vjk")
            V.tensor_tensor(VJK[:], PJPK,
                            _bb(CI, [[0, 2], [1, 3 * nb]]),
                            ALU.subtract)
            SQP = geo.tile([128, 9 * nb], f32, tag="sqp")
            S.activation(SQP[:, 0:6 * nb], VJK[:], AF.Square)
            V.tensor_tensor(SQP[:, 6 * nb:9 * nb], VJK[:, 0:3 * nb],
                            VJK[:, 3 * nb:6 * nb], ALU.mult)

            rv = geo.tile([128, RG * 72], f32, tag="rv")
            V.tensor_tensor(
                _bb(rv[:], [[72, RG], [24, 3], [1, 24]]),
                _bb(CJ, [[3, RG], [1, 3], [0, 24]]),
                _bb(CB, [[72, RG], [24, 3], [1, 24]]),
                ALU.subtract)
            rvs = geo.tile([128, RG * 72], f32, tag="rvs")
            S.activation(rvs[:], rv[:], AF.Square)

            T1 = geo.tile([128, 3 * nb], f32, tag="t1")
            V.tensor_tensor(_bb(T1[:], [[nb, 3], [1, nb]]),
                            _bb(SQP[:], [[3 * nb, 3], [3, nb]], off=0),
                            _bb(SQP[:], [[3 * nb, 3], [3, nb]], off=1),
                            ALU.add)
            D2 = geo.tile([128, 3 * nb], f32, tag="d2")
            V.tensor_tensor(_bb(D2[:], [[nb, 3], [1, nb]]),
                            _bb(T1[:], [[nb, 3], [1, nb]]),
                            _bb(SQP[:], [[3 * nb, 3], [3, nb]], off=2),
                            ALU.add)

            L2 = geo.tile([128, 2 * nb], f32, tag="l2")
            S.activation(L2[:], D2[:, 0:2 * nb], AF.Ln)
            DD = geo.tile([128, 2 * nb], f32, tag="dd")
            S.activation(DD[:], L2[:], AF.Exp, scale=0.5)
            RR = geo.tile([128, 2 * nb], f32, tag="rr")
            S.activation(RR[:], L2[:], AF.Exp, scale=-0.5)

            rt1 = geo.tile([128, RG * 24], f32, tag="rt1")
            V.tensor_tensor(rt1[:],
                            _bb(rvs[:], [[72, RG], [1, 24]], off=0),
                            _bb(rvs[:], [[72, RG], [1, 24]], off=24),
                            ALU.add)
            rd2 = geo.tile([128, RG * 24], f32, tag="rd2")
            V.tensor_tensor(rd2[:], rt1[:],
                            _bb(rvs[:], [[72, RG], [1, 24]], off=48),
                            ALU.add)
            rln = geo.tile([128, RG * 24], f32, tag="rln")
            S.activation(rln[:], rd2[:], AF.Ln)
            rdist = geo.tile([128, RG * 24], f32, tag="rdist")
            S.activation(rdist[:], rln[:], AF.Exp, scale=0.5)

            # cos/sin: Q = [cos | sin]
            Q = geo.tile([128, 2 * nb], f32, tag="q")
            rjrk = geo.tile([128, nb], f32, tag="rjrk")
            V.tensor_tensor(rjrk[:], RR[:, 0:nb], RR[:, nb:2 * nb], ALU.mult)
            V.scalar_tensor_tensor(Q[:, 0:nb], D2[:, 2 * nb:3 * nb], 0.95,
                                   rjrk[:], ALU.mult, ALU.mult)
            c2 = geo.tile([128, nb], f32, tag="c2")
            S.activation(c2[:], Q[:, 0:nb], AF.Square)
            sln = geo.tile([128, nb], f32, tag="sln")
            S.activation(sln[:], c2[:], AF.Ln, scale=-1.0, bias=bias(_C_ONE))
            S.activation(Q[:, nb:2 * nb], sln[:], AF.Exp, scale=0.5)

            # merged cutoff poly: U = [uj|uk (ang) | ur (rad)]
            NU = 2 * nb + RG * 24
            U = geo.tile([128, NU], f32, tag="u")
            V.tensor_scalar(U[:, 0:2 * nb], DD[:], RCA, 1.0 / RCA, ALU.min,
                            ALU.mult)
            S.activation(U[:, 2 * nb:NU], rln[:], AF.Exp, scale=0.5,
                         bias=bias(_C_NLRCR))
            UV = geo.tile([128, NU], f32, tag="uv")
            S.activation(UV[:], U[:], AF.Square)
            acc = geo.tile([128, NU], f32, tag="acc")
            V.scalar_tensor_tensor(acc[:], UV[:], float(a[3]), UV[:],
                                   ALU.add, ALU.mult)
            V.scalar_tensor_tensor(acc[:], acc[:], float(a[2]), UV[:],
                                   ALU.add, ALU.mult)
            V.scalar_tensor_tensor(acc[:], acc[:], float(a[1]), UV[:],
                                   ALU.add, ALU.mult)
            V.tensor_scalar(acc[:], acc[:], float(a[0]), None, ALU.add)
            wm = geo.tile([128, nb], f32, tag="wm")
            V.tensor_tensor(wm[:], acc[:, 0:nb], acc[:, nb:2 * nb], ALU.mult)
            w2 = geo.tile([128, nb], f32, tag="w2")
            S.activation(w2[:], wm[:], AF.Square)
            rfc = geo.tile([128, RG * 24], f32, tag="rfc")
            S.activation(rfc[:], acc[:, 2 * nb:NU], AF.Square)

            usum = geo.tile([128, nb], f32, tag="usum")
            V.tensor_tensor(usum[:], DD[:, 0:nb], DD[:, nb:2 * nb], ALU.add)

            # radial rt fills the V gap while S runs the half-0 ACT chain
            rt = feat.tile([128, RG * 384], f32, tag="rt")
            rsq = feat.tile([128, RG * 384], f32, tag="rsq")
            rex = feat.tile([128, RG * 384], f32, tag="rex")
            rad = feat.tile([128, RG * 384], bf16, tag="rad")

            t4 = feat.tile([128, 4 * nb], f32, tag="t4")
            t4s = feat.tile([128, 4 * nb], f32, tag="t4s")
            f2 = feat.tile([128, 4 * nb], f32, tag="f2")
            wf2 = feat.tile([128, 4 * nb], f32, tag="wf2")
            AB = feat.tile([128, 8 * nb], f32, tag="AB")
            q8 = feat.tile([128, 8 * nb], f32, tag="q8")
            lnq = feat.tile([128, 8 * nb], f32, tag="lnq")
            f1 = feat.tile([128, 8 * nb], f32, tag="f1")

            gwmax = max(chi[min(gt + PGRP, NG) - 1] - clo[gt] + 1
                        for gt in range(0, NG, PGRP))

            def ang_features(cah, cbh):
                rng = cbh - cah
                V.tensor_tensor(
                    _bb(t4[:], [[4, rng], [1, 4]], off=4 * cah),
                    _bb(usum[:], [[1, rng], [0, 4]], off=cah),
                    _bb(CT[:, _C_SHF2A:], [[0, rng], [1, 4]]),
                    ALU.subtract)
                S.activation(t4s[:, 4 * cah:4 * cbh], t4[:, 4 * cah:4 * cbh],
                             AF.Square)
                S.activation(f2[:, 4 * cah:4 * cbh], t4s[:, 4 * cah:4 * cbh],
                             AF.Exp, scale=-ETA_A / 4.0, bias=bias(_C_F2B))
                V.tensor_tensor(
                    _bb(wf2[:], [[4, rng], [1, 4]], off=4 * cah),
                    _bb(w2[:], [[1, rng], [0, 4]], off=cah),
                    _bb(f2[:], [[4, rng], [1, 4]], off=4 * cah),
                    ALU.mult)
                V.tensor_tensor(
                    _bb(AB[:], [[4 * nb, 2], [4, rng], [1, 4]], off=4 * cah),
                    _bb(Q[:], [[nb, 2], [1, rng], [0, 4]], off=cah),
                    _bb(CT[:, _C_CZSZ:], [[4, 2], [0, rng], [1, 4]]),
                    ALU.mult)
                V.tensor_tensor(
                    _bb(q8[:], [[8, rng], [1, 4]], off=8 * cah),
                    _bb(AB[:], [[4, rng], [1, 4]], off=4 * cah),
                    _bb(AB[:], [[4, rng], [1, 4]], off=4 * nb + 4 * cah),
                    ALU.add)
                V.tensor_tensor(
                    _bb(q8[:], [[8, rng], [1, 4]], off=8 * cah + 4),
                    _bb(AB[:], [[4, rng], [-1, 4]], off=4 * nb + 4 * cah + 3),
                    _bb(AB[:], [[4, rng], [-1, 4]], off=4 * cah + 3),
                    ALU.subtract)
                S.activation(lnq[:, 8 * cah:8 * cbh], q8[:, 8 * cah:8 * cbh],
                             AF.Ln, bias=bias(_C_HALF))
                S.activation(f1[:, 8 * cah:8 * cbh], lnq[:, 8 * cah:8 * cbh],
                             AF.Exp, scale=float(ZETA))

            def ang_emit(bi):
                gt = bi * PGRP
                gl = min(gt + PGRP, NG)
                ca, cb = clo[gt], chi[gl - 1] + 1
                Gt = gp.tile([128, 32 * gwmax], bf16, tag="G")
                V.tensor_tensor(
                    _bb(Gt[:], [[32, cb - ca], [8, 4], [1, 8]]),
                    _bb(wf2[:, 4 * ca:], [[4, cb - ca], [1, 4], [0, 8]]),
                    _bb(f1[:, 8 * ca:], [[8, cb - ca], [0, 4], [1, 8]]),
                    ALU.mult)
                pt = ps.tile([GSEG, 32 * PGRP], f32, tag="ps")
                for g in range(gt, gl):
                    gi = g - gt
                    for k in range(span[g]):
                        cc = clo[g] + k
                        nc.tensor.matmul(
                            pt[:, 32 * gi:32 * (gi + 1)],
                            B8[:, OHO + GSEG * (mm_base[g] + k):
                                  OHO + GSEG * (mm_base[g] + k + 1)],
                            Gt[:, 32 * (cc - ca):32 * (cc - ca + 1)],
                            start=(k == 0), stop=(k == span[g] - 1))
                sl = slice(32 * gt, 32 * gl)
                if bi % 2 == 0:
                    V.tensor_scalar(AZSTG[:, sl], pt[:], 0.0, None, ALU.add)
                else:
                    S.activation(AZSTG[:, sl], pt[:], AF.Copy)
                nc.sync.dma_start(outa_d[:, sl], AZSTG[:, sl])

            def rad_gauss(h):
                HH = RG * 384 // 2
                sl = slice(h * HH, (h + 1) * HH)
                V.tensor_tensor(
                    _bb(rt[:], [[384, RG // 2], [16, 24], [1, 16]],
                        off=h * HH),
                    _bb(rdist[:], [[24, RG // 2], [1, 24], [0, 16]],
                        off=h * RG * 24 // 2),
                    _bb(CT[:, _C_SHFR:], [[0, RG // 2], [0, 24], [1, 16]]),
                    ALU.subtract)
                S.activation(rsq[:, sl], rt[:, sl], AF.Square)
                S.activation(rex[:, sl], rsq[:, sl], AF.Exp, scale=-ETA_R,
                             bias=bias(_C_RADB))
                V.tensor_tensor(
                    _bb(rad[:], [[384, RG // 2], [16, 24], [1, 16]],
                        off=h * HH),
                    _bb(rfc[:], [[24, RG // 2], [1, 24], [0, 16]],
                        off=h * RG * 24 // 2),
                    _bb(rex[:], [[384, RG // 2], [16, 24], [1, 16]],
                        off=h * HH),
                    ALU.mult)

            # half 0 features -> PE blocks 0,1 while half 1 + radial follow
            ang_features(clo[0], chi[2 * PGRP - 1] + 1)
            rad_gauss(0)
            ang_emit(0)
            ang_emit(1)
            ang_features(clo[2 * PGRP], chi[NG - 1] + 1)
            rad_gauss(1)
            ang_emit(2)
            ang_emit(3)

            # radial binning matmuls: 4-up col-tiled into 2 PSUM banks
            rpt0 = psr.tile([128, 384], f32, tag="rps0")
            rpt1 = psr.tile([128, 384], f32, tag="rps1")
            for g in range(RG):
                pt = rpt0 if g < 4 else rpt1
                j = g % 4
                nc.tensor.matmul(pt[32 * j:32 * (j + 1), :],
                                 RSP[:, RSEGW * g:RSEGW * (g + 1)],
                                 rad[:, 384 * g:384 * (g + 1)],
                                 start=True, stop=True,
                                 tile_position=(0, 32 * j))
            V.tensor_scalar(RDSTG[:, 0:384], rpt0[:], 0.0, None, ALU.add)
            S.activation(RDSTG[:64, 384:768], rpt1[:64, :], AF.Copy)
            nc.sync.dma_start(outr_d[:, 0:384], RDSTG[:, 0:384])
            nc.sync.dma_start(outr_d[:64, 384:768], RDSTG[:64, 384:768])

    _patch_act_tables()
    nc.compile()
    return nc


_ACT_PATCHED = False


def _patch_act_tables():
    """Restrict every activation fn to the natural_log_exp table set so the
    table-load pass emits exactly ONE load."""
    global _ACT_PATCHED
    if _ACT_PATCHED:
        return
    orig = bacc.get_activation_tables

    def patched(arch):
        t = dict(orig(arch))
        out = {}
        for name, fns in t.items():
            if name != "natural_log_exp_and_others":
                fns = set()
            out[name] = fns
        return out

    bacc.get_activation_tables = patched
    _ACT_PATCHED = True


_CACHE = {}


def _decode(res, meta):
    s0g = meta["s0g"]
    out = np.empty((M, A, 384), np.float32)
    for c in range(NCORES):
        outa = np.asarray(res.results[c]["outa"]).astype(np.float32)
        outr = np.asarray(res.results[c]["outr"]).astype(np.float32)
        ang = outa.reshape(GSEG, NG, 32)[:120]
        ang = ang.reshape(10, 12, MLOC, 2, 32)
        ang = ang.transpose(2, 3, 1, 0, 4).reshape(MLOC, A, 320)
        rad = np.zeros((MLOC, A, 4, 16), np.float32)
        for g in range(RG):
            bank = outr[:, 384 * (g // 4):384 * (g // 4 + 1)]
            blk = bank[32 * (g % 4):32 * (g % 4) + 32]
            blk = blk.reshape(8, 4, 24, 16)
            smax = min(MLOC - s0g[g], 8)
            rad[s0g[g]:s0g[g] + smax] += blk[:smax].transpose(0, 2, 1, 3)
        mols = meta["slot2mol"][c]
        out[mols, :, :64] = rad.reshape(MLOC, A, 64)
        out[mols, :, 64:] = ang
    return out


def kernel(species, coordinates, coefficients=None):
    species = np.asarray(species)
    coordinates = np.asarray(coordinates, np.float32)
    meta, arrays = _prep(species, coordinates)
    key = (meta["nch"], meta["clo"], meta["chi"])
    if key not in _CACHE:
        _CACHE[key] = _build(meta["nch"], list(meta["clo"]),
                             list(meta["chi"]))
    nc = _CACHE[key]

    in_maps = []
    for c in range(NCORES):
        in_maps.append({"f32in": arrays["f32in"][c],
                        "bf8in": arrays["bf8in"][c]})
    res = run_bass_kernel_spmd(nc, in_maps, core_ids=list(range(NCORES)))
    return _decode(res, meta)


# revision 12
# speedup vs baseline: 1.1543x; 1.1543x over previous
"""ANI-style AEV computer (radial + angular) on 8 Trainium2 NeuronCores.

Strategy
--------
Data-parallel over molecules (32/core), host-side *indexing only*; all AEV
float math runs on-device.

Angular: host enumerates surviving triples (center i, neighbors j<k within
Rca) into a flat per-core list sorted by (slot, half, center, pair-bin).
Device computes geometry -> cutoffs -> f2/f1 -> G (bf16, 32 feats) per
128-triple chunk and bins G into (center, species-pair) segments with PE
matmuls against per-chunk one-hot matrices (fp8 stationary, PSUM-accum
per group).  The feature chain is split into two chunk-halves so the PE
binning of half 0 overlaps the DVE/ACT work of half 1.

Radial: dense over all (i,j) pairs, 768 rows packed as 6x128 partitions,
species-binned with small block one-hot matmuls packed 4-up into a PSUM
bank via tile_position col-tiling.

Engine split: DVE runs every two-input elementwise op; ACT runs every
unary op (ln/exp/square, with scale+bias folds such as sin via
ln(1-cos^2)).  GpSimd is NOT used: its SBUF port is the same physical
port DVE needs for two-input ops (exclusive full-instruction lock).
Self-pair masking is done by offsetting diagonal coordinates host-side.
Inputs arrive as TWO contiguous DMAs (f32 pack on the ACT queue, fp8
pack on the sync queue) to minimize serial descriptor generation.  One
ACT table set serves every activation.  Outputs in bf16.
"""

import os
import sys

import numpy as np

for _p in ("/opt/trn_rl_repo", "/root/.axon_site/_ro/trn_rl_repo"):
    if os.path.isdir(_p) and _p not in sys.path:
        sys.path.insert(0, _p)

import concourse.bass as bass
import concourse.mybir as mybir
from concourse import bacc, tile
from concourse.bass_utils import run_bass_kernel_spmd

import ml_dtypes

AF = mybir.ActivationFunctionType
ALU = mybir.AluOpType
dt = mybir.dt
AP = bass.AP
FP8 = ml_dtypes.float8_e4m3

# ---- hyperparameters (match reference) ----
NCORES = 8
M, A = 256, 24
MLOC = M // NCORES          # 32 molecules per core
RCR, RCA = 5.2, 3.5
ETA_R, ETA_A, ZETA = 16.0, 8.0, 32.0
SHF_R = np.linspace(0.9, 5.2, 17)[:-1].astype(np.float64)   # 16
SHF_A = np.linspace(0.9, 3.5, 5)[:-1].astype(np.float64)    # 4
SHF_Z = (np.arange(8) + 0.5) * np.pi / 8.0                   # 8
NPAIR, RSUB, ASUB = 10, 16, 32
NSEG = 120
GSEG = 128                  # one-hot width
NG = 2 * MLOC               # 64 groups/core
PGRP = 16                   # psum groups per PSUM bank tile
RG = 6                      # radial groups: 768 rows = 6 x 128
RSEGW = 32                  # radial one-hot width per group

_TRIU = np.zeros((4, 4), np.int64)
_s1, _s2 = np.triu_indices(4)
_TRIU[_s1, _s2] = np.arange(len(_s1))
_TRIU[_s2, _s1] = _TRIU[_s1, _s2]

# ---- degree-4 (in v=u^2) Chebyshev fit of cos(pi*u/2) on u in [0,1] ----
def _cos_poly():
    v = np.linspace(0.0, 1.0, 4001)
    tgt = np.cos(0.5 * np.pi * np.sqrt(v))
    from numpy.polynomial import chebyshev as C
    ch = C.Chebyshev.fit(v, tgt, 4, domain=[0, 1])
    pw = ch.convert(kind=np.polynomial.Polynomial)
    c = pw.coef
    K = c[4]
    a = c[:4] / K
    return K, a

_POLY_K, _POLY_A = _cos_poly()

# const column map (first 33 cols of the f32 input pack)
_C_SHF2A = 0     # 4  : 2*shf_a
_C_SHFR = 4      # 16 : shf_r
_C_CZSZ = 20     # 8  : 0.5*cos(shf_z[0:4]) | 0.5*sin(shf_z[0:4])
_C_F2B = 28      # 1  : angular exp bias ln(2*K^4)
_C_RADB = 29     # 1  : radial exp bias ln(0.25*K^2)
_C_HALF = 30     # 1  : 0.5
_C_NLRCR = 31    # 1  : -ln(RCR)
_C_ONE = 32      # 1  : 1.0
_C_W = 33


def _build_consts():
    ct = np.zeros((128, _C_W), np.float32)
    ct[:, _C_SHF2A:_C_SHF2A + 4] = 2.0 * SHF_A
    ct[:, _C_SHFR:_C_SHFR + 16] = SHF_R
    ct[:, _C_CZSZ:_C_CZSZ + 4] = 0.5 * np.cos(SHF_Z[:4])
    ct[:, _C_CZSZ + 4:_C_CZSZ + 8] = 0.5 * np.sin(SHF_Z[:4])
    K = _POLY_K
    ct[:, _C_F2B] = np.log(2.0) + 4.0 * np.log(abs(K))
    ct[:, _C_RADB] = np.log(0.25) + 2.0 * np.log(abs(K))
    ct[:, _C_HALF] = 0.5
    ct[:, _C_NLRCR] = -np.log(RCR)
    ct[:, _C_ONE] = 1.0
    return ct


# ============================================================
# host-side indexing prep
# ============================================================

def _prep(species, coordinates):
    sp = np.asarray(species)
    co = np.asarray(coordinates, np.float32)
    cod = co.astype(np.float64)
    vec = cod[:, None, :, :] - cod[:, :, None, :]
    dmat = np.sqrt(np.maximum((vec ** 2).sum(-1), 0.0))
    adj = (dmat <= RCA) & ~np.eye(A, dtype=bool)[None]

    nbrs = [[np.where(adj[m, i])[0] for i in range(A)] for m in range(M)]
    tri_mi = np.array([[len(nbrs[m][i]) * (len(nbrs[m][i]) - 1) // 2
                        for i in range(A)] for m in range(M)], np.int64)
    Th = np.stack([tri_mi[:, :12].sum(1), tri_mi[:, 12:].sum(1)], 1)

    order = np.argsort(-(Th.sum(1)), kind="stable")
    slot2mol = np.empty((NCORES, MLOC), np.int64)
    for s in range(MLOC):
        for c in range(NCORES):
            slot2mol[c, s] = order[s * NCORES + c]

    SYNCW = 4
    tlo = np.zeros((NCORES, NG), np.int64)
    thi = np.zeros((NCORES, NG), np.int64)
    posv = np.zeros(NCORES, np.int64)
    for g in range(NG):
        s, h = g // 2, g % 2
        if g % SYNCW == 0:
            posv[:] = int(np.ceil(posv.max() / 128.0)) * 128
        tlo[:, g] = posv
        posv += Th[slot2mol[:, s], h]
        thi[:, g] = posv
    nch = int(np.ceil(posv.max() / 128.0))
    clo = np.empty(NG, np.int64)
    chi = np.empty(NG, np.int64)
    for g in range(NG):
        clo[g] = (tlo[:, g] // 128).min()
        hi = np.maximum(thi[:, g] - 1, tlo[:, g]) // 128
        chi[g] = max(hi.max(), clo[g])
    span = (chi - clo + 1).astype(np.int64)
    mm_base = np.concatenate([[0], np.cumsum(span)])
    n_mm = int(mm_base[-1])

    # f32 input pack: [consts(33) | pj|pk|ci (9*nch) | radf (450)]
    NF = _C_W + 9 * nch + RG * 75
    f32in = np.zeros((NCORES, 128, NF), np.float32)
    f32in[:, :, :_C_W] = _build_consts()
    o = _C_W
    pj = f32in[:, :, o:o + 3 * nch].reshape(NCORES, 128, nch, 3)
    pk = f32in[:, :, o + 3 * nch:o + 6 * nch].reshape(NCORES, 128, nch, 3)
    ci = f32in[:, :, o + 6 * nch:o + 9 * nch].reshape(NCORES, 128, nch, 3)
    o += 9 * nch
    rcj = f32in[:, :, o:o + RG * 3].reshape(NCORES, 128, RG, 3)
    rcb = f32in[:, :, o + RG * 3:].reshape(NCORES, 128, RG, 3, 24)

    # fp8 input pack: [rsp (RG*32) | oh (n_mm*128)]
    bf8in = np.zeros((NCORES, 128, RG * RSEGW + n_mm * GSEG), FP8)
    rsp = bf8in[:, :, :RG * RSEGW]
    oh = bf8in[:, :, RG * RSEGW:].reshape(NCORES, 128, n_mm, GSEG)

    for c in range(NCORES):
        def put_pad(a, b, mref):
            if a >= b:
                return
            t_idx = np.arange(a, b)
            chs, ts = t_idx // 128, t_idx % 128
            pj[c, ts, chs] = mref + np.array([50, 0, 0], np.float32)
            pk[c, ts, chs] = mref + np.array([0, 50, 0], np.float32)
            ci[c, ts, chs] = mref
        prev_end = 0
        for s in range(MLOC):
            m = slot2mol[c, s]
            for h in range(2):
                g = 2 * s + h
                put_pad(prev_end, tlo[c, g], co[m, 0])
                pos = tlo[c, g]
                for u in range(12):
                    i = h * 12 + u
                    nb = nbrs[m][i]
                    if len(nb) < 2:
                        continue
                    jj, kk = np.triu_indices(len(nb), 1)
                    j, k = nb[jj], nb[kk]
                    p = _TRIU[sp[m, j], sp[m, k]]
                    o2 = np.argsort(p, kind="stable")
                    j, k, p = j[o2], k[o2], p[o2]
                    n = len(j)
                    t_idx = np.arange(pos, pos + n)
                    chs, ts = t_idx // 128, t_idx % 128
                    pj[c, ts, chs] = co[m, j]
                    pk[c, ts, chs] = co[m, k]
                    ci[c, ts, chs] = np.broadcast_to(co[m, i], (n, 3))
                    oh[c, ts, mm_base[g] + chs - clo[g], p * 12 + u] = 1
                    pos += n
                prev_end = pos
        put_pad(prev_end, nch * 128, co[slot2mol[c, 0], 0])

    s0g = [(128 * g) // 24 for g in range(RG)]
    for c in range(NCORES):
        for g in range(RG):
            for p in range(128):
                r = g * 128 + p
                s, j = r // 24, r % 24
                m = slot2mol[c, s]
                rcj[c, p, g] = co[m, j]
                cb = co[m].T.copy()
                cb[:, j] += 10.0                 # self-pair offset
                rcb[c, p, g] = cb
                rsp[c, p, g * RSEGW + (s - s0g[g]) * 4 + sp[m, j]] = 1

    meta = dict(nch=nch, n_mm=n_mm, clo=tuple(int(x) for x in clo),
                chi=tuple(int(x) for x in chi), slot2mol=slot2mol,
                s0g=s0g)
    arrays = dict(f32in=f32in, bf8in=bf8in)
    return meta, arrays


# ============================================================
# device program
# ============================================================

def _bb(ap, dims, off=0):
    return AP(ap.tensor, ap.offset + off,
              [list(ap.ap[0])] + [list(d) for d in dims])


def _build(nch, clo, chi):
    span = [chi[g] - clo[g] + 1 for g in range(NG)]
    mm_base = [0]
    for g in range(NG):
        mm_base.append(mm_base[-1] + span[g])
    n_mm = mm_base[-1]
    nb = nch
    NF = _C_W + 9 * nch + RG * 75
    NB8 = RG * RSEGW + n_mm * GSEG

    nc = bacc.Bacc(None, target_bir_lowering=False)
    f32_d = nc.declare_dram_parameter("f32in", [128, NF], dt.float32, False)
    bf8_d = nc.declare_dram_parameter("bf8in", [128, NB8], dt.float8e4,
                                      False)
    outa_d = nc.declare_dram_parameter("outa", [GSEG, NG * 32], dt.bfloat16,
                                       True)
    outr_d = nc.declare_dram_parameter("outr", [128, 2 * 24 * 16],
                                       dt.bfloat16, True)

    a = _POLY_A
    f32, bf16 = dt.float32, dt.bfloat16

    with tile.TileContext(nc) as tc:
        with (
            tc.tile_pool(name="io", bufs=1) as io,
            tc.tile_pool(name="geo", bufs=1) as geo,
            tc.tile_pool(name="feat", bufs=1) as feat,
            tc.tile_pool(name="stg", bufs=1) as stg,
            tc.tile_pool(name="gp", bufs=3) as gp,
            tc.tile_pool(name="ps", bufs=4, space="PSUM") as ps,
            tc.tile_pool(name="psr", bufs=2, space="PSUM") as psr,
        ):
            V = nc.vector
            S = nc.scalar

            # all input DMAs on ONE queue (sync) so descriptor order == the
            # priority order: the f32 pack gates every compute op, the
            # one-hot chunks are only needed by the (late) PE phase.
            F32 = io.tile([128, NF], f32, tag="f32in")
            nc.sync.dma_start(F32[:], f32_d[:])
            B8 = io.tile([128, NB8], dt.float8e4, tag="bf8in")
            nc.sync.dma_start(B8[:, 0:RG * RSEGW], bf8_d[:, 0:RG * RSEGW])

            CT = F32
            AOFF = _C_W
            ROFF = _C_W + 9 * nch
            PJPK = F32[:, AOFF:AOFF + 6 * nch]
            CI = F32[:, AOFF + 6 * nch:AOFF + 9 * nch]
            CJ = F32[:, ROFF:ROFF + RG * 3]
            CB = F32[:, ROFF + RG * 3:ROFF + RG * 75]
            RSP = B8[:, 0:RG * RSEGW]
            OHO = RG * RSEGW
            # one-hot in 4 chunks aligned to the PGRP blocks
            ohsplit = [mm_base[min(b * PGRP, NG)]
                       for b in range(NG // PGRP + 1)]
            for b in range(NG // PGRP):
                lo, hi = ohsplit[b], ohsplit[b + 1]
                if hi > lo:
                    nc.sync.dma_start(
                        B8[:, OHO + GSEG * lo:OHO + GSEG * hi],
                        bf8_d[:, OHO + GSEG * lo:OHO + GSEG * hi])

            AZSTG = stg.tile([GSEG, NG * 32], bf16)
            RDSTG = stg.tile([128, 768], bf16)

            def bias(col):
                return CT[:, col:col + 1]

            # ================= geometry (V + S) =================
            VJK = geo.tile([128, 6 * nb], f32, tag="vjk")
            V.tensor_tensor(VJK[:], PJPK,
                            _bb(CI, [[0, 2], [1, 3 * nb]]),
                            ALU.subtract)
            SQP = geo.tile([128, 9 * nb], f32, tag="sqp")
            S.activation(SQP[:, 0:6 * nb], VJK[:], AF.Square)
            V.tensor_tensor(SQP[:, 6 * nb:9 * nb], VJK[:, 0:3 * nb],
                            VJK[:, 3 * nb:6 * nb], ALU.mult)

            rv = geo.tile([128, RG * 72], f32, tag="rv")
            V.tensor_tensor(
                _bb(rv[:], [[72, RG], [24, 3], [1, 24]]),
                _bb(CJ, [[3, RG], [1, 3], [0, 24]]),
                _bb(CB, [[72, RG], [24, 3], [1, 24]]),
                ALU.subtract)
            rvs = geo.tile([128, RG * 72], f32, tag="rvs")
            S.activation(rvs[:], rv[:], AF.Square)

            T1 = geo.tile([128, 3 * nb], f32, tag="t1")
            V.tensor_tensor(_bb(T1[:], [[nb, 3], [1, nb]]),
                            _bb(SQP[:], [[3 * nb, 3], [3, nb]], off=0),
                            _bb(SQP[:], [[3 * nb, 3], [3, nb]], off=1),
                            ALU.add)
            D2 = geo.tile([128, 3 * nb], f32, tag="d2")
            V.tensor_tensor(_bb(D2[:], [[nb, 3], [1, nb]]),
                            _bb(T1[:], [[nb, 3], [1, nb]]),
                            _bb(SQP[:], [[3 * nb, 3], [3, nb]], off=2),
                            ALU.add)

            L2 = geo.tile([128, 2 * nb], f32, tag="l2")
            S.activation(L2[:], D2[:, 0:2 * nb], AF.Ln)
            DD = geo.tile([128, 2 * nb], f32, tag="dd")
            S.activation(DD[:], L2[:], AF.Exp, scale=0.5)
            RR = geo.tile([128, 2 * nb], f32, tag="rr")
            S.activation(RR[:], L2[:], AF.Exp, scale=-0.5)

            rt1 = geo.tile([128, RG * 24], f32, tag="rt1")
            V.tensor_tensor(rt1[:],
                            _bb(rvs[:], [[72, RG], [1, 24]], off=0),
                            _bb(rvs[:], [[72, RG], [1, 24]], off=24),
                            ALU.add)
            rd2 = geo.tile([128, RG * 24], f32, tag="rd2")
            V.tensor_tensor(rd2[:], rt1[:],
                            _bb(rvs[:], [[72, RG], [1, 24]], off=48),
                            ALU.add)
            rln = geo.tile([128, RG * 24], f32, tag="rln")
            S.activation(rln[:], rd2[:], AF.Ln)
            rdist = geo.tile([128, RG * 24], f32, tag="rdist")
            S.activation(rdist[:], rln[:], AF.Exp, scale=0.5)

            # cos/sin: Q = [cos | sin]
            Q = geo.tile([128, 2 * nb], f32, tag="q")
            rjrk = geo.tile([128, nb], f32, tag="rjrk")
            V.tensor_tensor(rjrk[:], RR[:, 0:nb], RR[:, nb:2 * nb], ALU.mult)
            V.scalar_tensor_tensor(Q[:, 0:nb], D2[:, 2 * nb:3 * nb], 0.95,
                                   rjrk[:], ALU.mult, ALU.mult)
            c2 = geo.tile([128, nb], f32, tag="c2")
            S.activation(c2[:], Q[:, 0:nb], AF.Square)
            sln = geo.tile([128, nb], f32, tag="sln")
            S.activation(sln[:], c2[:], AF.Ln, scale=-1.0, bias=bias(_C_ONE))
            S.activation(Q[:, nb:2 * nb], sln[:], AF.Exp, scale=0.5)

            # merged cutoff poly: U = [uj|uk (ang) | ur (rad)]
            NU = 2 * nb + RG * 24
            U = geo.tile([128, NU], f32, tag="u")
            V.tensor_scalar(U[:, 0:2 * nb], DD[:], RCA, 1.0 / RCA, ALU.min,
                            ALU.mult)
            S.activation(U[:, 2 * nb:NU], rln[:], AF.Exp, scale=0.5,
                         bias=bias(_C_NLRCR))
            UV = geo.tile([128, NU], f32, tag="uv")
            S.activation(UV[:], U[:], AF.Square)
            acc = geo.tile([128, NU], f32, tag="acc")
            V.scalar_tensor_tensor(acc[:], UV[:], float(a[3]), UV[:],
                                   ALU.add, ALU.mult)
            V.scalar_tensor_tensor(acc[:], acc[:], float(a[2]), UV[:],
                                   ALU.add, ALU.mult)
            V.scalar_tensor_tensor(acc[:], acc[:], float(a[1]), UV[:],
                                   ALU.add, ALU.mult)
            V.tensor_scalar(acc[:], acc[:], float(a[0]), None, ALU.add)
            wm = geo.tile([128, nb], f32, tag="wm")
            V.tensor_tensor(wm[:], acc[:, 0:nb], acc[:, nb:2 * nb], ALU.mult)
            w2 = geo.tile([128, nb], f32, tag="w2")
            S.activation(w2[:], wm[:], AF.Square)
            rfc = geo.tile([128, RG * 24], f32, tag="rfc")
            S.activation(rfc[:], acc[:, 2 * nb:NU], AF.Square)

            usum = geo.tile([128, nb], f32, tag="usum")
            V.tensor_tensor(usum[:], DD[:, 0:nb], DD[:, nb:2 * nb], ALU.add)

            # radial rt fills the V gap while S runs the half-0 ACT chain
            rt = feat.tile([128, RG * 384], f32, tag="rt")
            rsq = feat.tile([128, RG * 384], f32, tag="rsq")
            rex = feat.tile([128, RG * 384], f32, tag="rex")
            rad = feat.tile([128, RG * 384], bf16, tag="rad")

            t4 = feat.tile([128, 4 * nb], f32, tag="t4")
            t4s = feat.tile([128, 4 * nb], f32, tag="t4s")
            f2 = feat.tile([128, 4 * nb], f32, tag="f2")
            wf2 = feat.tile([128, 4 * nb], f32, tag="wf2")
            AB = feat.tile([128, 8 * nb], f32, tag="AB")
            q8 = feat.tile([128, 8 * nb], f32, tag="q8")
            lnq = feat.tile([128, 8 * nb], f32, tag="lnq")
            f1 = feat.tile([128, 8 * nb], f32, tag="f1")

            gwmax = max(chi[min(gt + PGRP, NG) - 1] - clo[gt] + 1
                        for gt in range(0, NG, PGRP))

            def ang_features(cah, cbh):
                rng = cbh - cah
                V.tensor_tensor(
                    _bb(t4[:], [[4, rng], [1, 4]], off=4 * cah),
                    _bb(usum[:], [[1, rng], [0, 4]], off=cah),
                    _bb(CT[:, _C_SHF2A:], [[0, rng], [1, 4]]),
                    ALU.subtract)
                S.activation(t4s[:, 4 * cah:4 * cbh], t4[:, 4 * cah:4 * cbh],
                             AF.Square)
                S.activation(f2[:, 4 * cah:4 * cbh], t4s[:, 4 * cah:4 * cbh],
                             AF.Exp, scale=-ETA_A / 4.0, bias=bias(_C_F2B))
                V.tensor_tensor(
                    _bb(wf2[:], [[4, rng], [1, 4]], off=4 * cah),
                    _bb(w2[:], [[1, rng], [0, 4]], off=cah),
                    _bb(f2[:], [[4, rng], [1, 4]], off=4 * cah),
                    ALU.mult)
                V.tensor_tensor(
                    _bb(AB[:], [[4 * nb, 2], [4, rng], [1, 4]], off=4 * cah),
                    _bb(Q[:], [[nb, 2], [1, rng], [0, 4]], off=cah),
                    _bb(CT[:, _C_CZSZ:], [[4, 2], [0, rng], [1, 4]]),
                    ALU.mult)
                V.tensor_tensor(
                    _bb(q8[:], [[8, rng], [1, 4]], off=8 * cah),
                    _bb(AB[:], [[4, rng], [1, 4]], off=4 * cah),
                    _bb(AB[:], [[4, rng], [1, 4]], off=4 * nb + 4 * cah),
                    ALU.add)
                V.tensor_tensor(
                    _bb(q8[:], [[8, rng], [1, 4]], off=8 * cah + 4),
                    _bb(AB[:], [[4, rng], [-1, 4]], off=4 * nb + 4 * cah + 3),
                    _bb(AB[:], [[4, rng], [-1, 4]], off=4 * cah + 3),
                    ALU.subtract)
                S.activation(lnq[:, 8 * cah:8 * cbh], q8[:, 8 * cah:8 * cbh],
                             AF.Ln, bias=bias(_C_HALF))
                S.activation(f1[:, 8 * cah:8 * cbh], lnq[:, 8 * cah:8 * cbh],
                             AF.Exp, scale=float(ZETA))

            def ang_emit(bi):
                gt = bi * PGRP
                gl = min(gt + PGRP, NG)
                ca, cb = clo[gt], chi[gl - 1] + 1
                Gt = gp.tile([128, 32 * gwmax], bf16, tag="G")
                V.tensor_tensor(
                    _bb(Gt[:], [[32, cb - ca], [8, 4], [1, 8]]),
                    _bb(wf2[:, 4 * ca:], [[4, cb - ca], [1, 4], [0, 8]]),
                    _bb(f1[:, 8 * ca:], [[8, cb - ca], [0, 4], [1, 8]]),
                    ALU.mult)
                pt = ps.tile([GSEG, 32 * PGRP], f32, tag="ps")
                for g in range(gt, gl):
                    gi = g - gt
                    for k in range(span[g]):
                        cc = clo[g] + k
                        nc.tensor.matmul(
                            pt[:, 32 * gi:32 * (gi + 1)],
                            B8[:, OHO + GSEG * (mm_base[g] + k):
                                  OHO + GSEG * (mm_base[g] + k + 1)],
                            Gt[:, 32 * (cc - ca):32 * (cc - ca + 1)],
                            start=(k == 0), stop=(k == span[g] - 1))
                sl = slice(32 * gt, 32 * gl)
                if bi % 2 == 0:
                    V.tensor_scalar(AZSTG[:, sl], pt[:], 0.0, None, ALU.add)
                else:
                    S.activation(AZSTG[:, sl], pt[:], AF.Copy)
                nc.sync.dma_start(outa_d[:, sl], AZSTG[:, sl])

            def rad_gauss(h):
                HH = RG * 384 // 2
                sl = slice(h * HH, (h + 1) * HH)
                V.tensor_tensor(
                    _bb(rt[:], [[384, RG // 2], [16, 24], [1, 16]],
                        off=h * HH),
                    _bb(rdist[:], [[24, RG // 2], [1, 24], [0, 16]],
                        off=h * RG * 24 // 2),
                    _bb(CT[:, _C_SHFR:], [[0, RG // 2], [0, 24], [1, 16]]),
                    ALU.subtract)
                S.activation(rsq[:, sl], rt[:, sl], AF.Square)
                S.activation(rex[:, sl], rsq[:, sl], AF.Exp, scale=-ETA_R,
                             bias=bias(_C_RADB))
                V.tensor_tensor(
                    _bb(rad[:], [[384, RG // 2], [16, 24], [1, 16]],
                        off=h * HH),
                    _bb(rfc[:], [[24, RG // 2], [1, 24], [0, 16]],
                        off=h * RG * 24 // 2),
                    _bb(rex[:], [[384, RG // 2], [16, 24], [1, 16]],
                        off=h * HH),
                    ALU.mult)

            # half 0 features -> PE blocks 0,1 while half 1 + radial follow
            ang_features(clo[0], chi[2 * PGRP - 1] + 1)
            rad_gauss(0)
            ang_emit(0)
            ang_emit(1)
            ang_features(clo[2 * PGRP], chi[NG - 1] + 1)
            rad_gauss(1)
            ang_emit(2)
            ang_emit(3)

            # radial binning matmuls: 4-up col-tiled into 2 PSUM banks
            rpt0 = psr.tile([128, 384], f32, tag="rps0")
            rpt1 = psr.tile([128, 384], f32, tag="rps1")
            for g in range(RG):
                pt = rpt0 if g < 4 else rpt1
                j = g % 4
                nc.tensor.matmul(pt[32 * j:32 * (j + 1), :],
                                 RSP[:, RSEGW * g:RSEGW * (g + 1)],
                                 rad[:, 384 * g:384 * (g + 1)],
                                 start=True, stop=True,
                                 tile_position=(0, 32 * j))
            V.tensor_scalar(RDSTG[:, 0:384], rpt0[:], 0.0, None, ALU.add)
            S.activation(RDSTG[:64, 384:768], rpt1[:64, :], AF.Copy)
            nc.sync.dma_start(outr_d[:, 0:384], RDSTG[:, 0:384])
            nc.sync.dma_start(outr_d[:64, 384:768], RDSTG[:64, 384:768])

    _patch_act_tables()
    nc.compile()
    return nc


_ACT_PATCHED = False


def _patch_act_tables():
    """Restrict every activation fn to the natural_log_exp table set so the
    table-load pass emits exactly ONE load."""
    global _ACT_PATCHED
    if _ACT_PATCHED:
        return
    orig = bacc.get_activation_tables

    def patched(arch):
        t = dict(orig(arch))
        out = {}
        for name, fns in t.items():
            if name != "natural_log_exp_and_others":
                fns = set()
            out[name] = fns
        return out

    bacc.get_activation_tables = patched
    _ACT_PATCHED = True


_CACHE = {}


def _decode(res, meta):
    s0g = meta["s0g"]
    out = np.empty((M, A, 384), np.float32)
    for c in range(NCORES):
        outa = np.asarray(res.results[c]["outa"]).astype(np.float32)
        outr = np.asarray(res.results[c]["outr"]).astype(np.float32)
        ang = outa.reshape(GSEG, NG, 32)[:120]
        ang = ang.reshape(10, 12, MLOC, 2, 32)
        ang = ang.transpose(2, 3, 1, 0, 4).reshape(MLOC, A, 320)
        rad = np.zeros((MLOC, A, 4, 16), np.float32)
        for g in range(RG):
            bank = outr[:, 384 * (g // 4):384 * (g // 4 + 1)]
            blk = bank[32 * (g % 4):32 * (g % 4) + 32]
            blk = blk.reshape(8, 4, 24, 16)
            smax = min(MLOC - s0g[g], 8)
            rad[s0g[g]:s0g[g] + smax] += blk[:smax].transpose(0, 2, 1, 3)
        mols = meta["slot2mol"][c]
        out[mols, :, :64] = rad.reshape(MLOC, A, 64)
        out[mols, :, 64:] = ang
    return out


def kernel(species, coordinates, coefficients=None):
    species = np.asarray(species)
    coordinates = np.asarray(coordinates, np.float32)
    meta, arrays = _prep(species, coordinates)
    key = (meta["nch"], meta["clo"], meta["chi"])
    if key not in _CACHE:
        _CACHE[key] = _build(meta["nch"], list(meta["clo"]),
                             list(meta["chi"]))
    nc = _CACHE[key]

    in_maps = []
    for c in range(NCORES):
        in_maps.append({"f32in": arrays["f32in"][c],
                        "bf8in": arrays["bf8in"][c]})
    res = run_bass_kernel_spmd(nc, in_maps, core_ids=list(range(NCORES)))
    return _decode(res, meta)
